# revision 13
# baseline (speedup 1.0000x reference)
"""DCT channel attention kernel for Trainium2 (8 NeuronCores, data-parallel over batch).

Math (per image b, channel c):
  Y = DH @ X @ DW^T              (2D orthonormal DCT of the 64x64 spatial map)
  energy = |Y[0,0]| + sum(top4(|Y| excluding DC))
  attn = sigmoid(relu(energy @ w1 + b1) @ w2 + b2)
  out = x * attn[:, :, None, None]

On-chip layout strategy per core (4 images = 2 partition-pairs):
  x loaded as [128 = (b2, h), C*W] tiles (free = (c, w), channel-major runs of 256B)
  M1: per channel-pair matmul, lhsT = X-slice [64 h, 128 (2ch, w)], rhs = DH^T
      -> A^T in PSUM [128 = (member, w), 64 i]      (fp32, exact)
  M2: lhsT = block-diag(DW^T, DW^T) [128, 128] stationary, rhs = A^T chunks
      -> Y^T [128 = (member, j), (pair, i)] in PSUM (float32r, 1 cyc/row)
  evict |Y| as bf16 (ScalarE Abs), DMA-flatten to [128 = channel, 4096 = (j, i)],
  DVE max (top-8 per partition) -> energy -> tiny MLP on PE -> attn broadcast via
  ones-outer-product matmul -> in-place DVE multiply -> store.

Channel permutation: flat row q within group g of 128 channels maps to true
channel c = g*128 + 2*(q % 64) + (q // 64). MLP weights are permuted host-side;
the broadcast matmul un-permutes via a strided AP.
"""

import numpy as np

B, C, H, W = 32, 256, 64, 64
NCORES = 8
BPC = B // NCORES  # images per core
CW = C * W
RED = 4
CH = C // RED  # 64 hidden units

# dtype for the second DCT matmul: "f32r" (fast, slightly reduced precision),
# "f32" (exact, 4 cyc/row), or "bf16"
M2_MODE = "f32r"
FLAT_BF16 = True


def _dct_matrix(N):
    n = np.arange(N, dtype=np.float64)
    k = np.arange(N, dtype=np.float64)[:, None]
    d = np.cos(np.pi * (2.0 * n + 1.0) * k / (2.0 * N))
    s = np.where(k == 0, np.sqrt(1.0 / N), np.sqrt(2.0 / N))
    return (d * s).astype(np.float32)  # [N, N], D[k, n]


def _perm_true_channel(g, q):
    # flat row q in group g -> true channel index
    return g * 128 + 2 * (q % 64) + (q // 64)


def build_nc(bpc=BPC):
    import concourse.bass as bass
    import concourse.tile as tile
    from concourse import bacc, mybir
    from contextlib import ExitStack

    f32 = mybir.dt.float32
    f32r = mybir.dt.float32r
    bf16 = mybir.dt.bfloat16
    flat_dt = bf16 if FLAT_BF16 else f32

    nc = bacc.Bacc("TRN2", target_bir_lowering=False, debug=False)

    xin = nc.dram_tensor("xin", [bpc, C, H, W], f32, kind="ExternalInput").ap()
    dht2_d = nc.dram_tensor("dht2", [128, 64], f32, kind="ExternalInput").ap()
    m2dt = {"f32r": f32, "f32": f32, "bf16": bf16}[M2_MODE]
    dwt2_d = nc.dram_tensor("dwt2", [128, 128], m2dt, kind="ExternalInput").ap()
    w1ps_d = nc.dram_tensor("w1ps", [128, 128], f32, kind="ExternalInput").ap()
    b1_d = nc.dram_tensor("b1v", [1, CH], f32, kind="ExternalInput").ap()
    w2p_d = nc.dram_tensor("w2p", [CH, 256], f32, kind="ExternalInput").ap()
    b2p_d = nc.dram_tensor("b2p", [1, 256], f32, kind="ExternalInput").ap()
    ident_d = nc.dram_tensor("ident", [128, 128], f32, kind="ExternalInput").ap()
    xout = nc.dram_tensor("xout", [bpc, C, H, W], f32, kind="ExternalOutput").ap()

    AF = mybir.ActivationFunctionType
    npairs = 2 if bpc > 2 else 1

    with tile.TileContext(nc) as tc, ExitStack() as ctx:
        const = ctx.enter_context(tc.tile_pool(name="const", bufs=1))
        xpool = ctx.enter_context(tc.tile_pool(name="xp", bufs=npairs))
        atsb = ctx.enter_context(tc.tile_pool(name="atsb", bufs=3))
        ypool = ctx.enter_context(tc.tile_pool(name="yab", bufs=2))
        flatp = ctx.enter_context(tc.tile_pool(name="flat", bufs=2))
        small = ctx.enter_context(tc.tile_pool(name="small", bufs=2))
        epool = ctx.enter_context(tc.tile_pool(name="energy", bufs=1))
        at_ps = ctx.enter_context(tc.tile_pool(name="atps", bufs=3, space="PSUM"))
        y_ps = ctx.enter_context(tc.tile_pool(name="yps", bufs=2, space="PSUM"))
        s_ps = ctx.enter_context(tc.tile_pool(name="sps", bufs=2, space="PSUM"))
        scrp = ctx.enter_context(tc.tile_pool(name="scr", bufs=2, space="DRAM"))

        # ---- constants ----
        dht2_t = const.tile([128, 64], f32)
        nc.sync.dma_start(dht2_t[:], dht2_d[:])
        dwt2_t = const.tile([128, 128], m2dt)
        nc.sync.dma_start(dwt2_t[:], dwt2_d[:])
        w1ps_t = const.tile([128, 128], f32)
        nc.sync.dma_start(w1ps_t[:], w1ps_d[:])
        b1_t = const.tile([1, CH], f32)
        nc.sync.dma_start(b1_t[:], b1_d[:])
        w2p_t = const.tile([CH, 256], f32)
        nc.sync.dma_start(w2p_t[:], w2p_d[:])
        b2p_t = const.tile([1, 256], f32)
        nc.sync.dma_start(b2p_t[:], b2p_d[:])
        ident_t = const.tile([128, 128], f32)
        nc.sync.dma_start(ident_t[:], ident_d[:])
        ones_t = const.tile([1, CH], f32)
        nc.vector.memset(ones_t[:], 1.0)
        # DVE-owned copies of every constant the PE reads: funnels all
        # matmul input deps onto the single DVE semaphore (the fp32
        # matmul LDW micro-op only supports one sync wait command).
        dht2c = const.tile([128, 64], f32)
        nc.vector.tensor_copy(dht2c[:], dht2_t[:])
        w1c = const.tile([128, 128], f32)
        nc.vector.tensor_copy(w1c[:], w1ps_t[:])
        b1c = const.tile([1, CH], f32)
        nc.vector.tensor_copy(b1c[:], b1_t[:])
        w2c = const.tile([CH, 256], f32)
        nc.vector.tensor_copy(w2c[:], w2p_t[:])
        b2c = const.tile([1, 256], f32)
        nc.vector.tensor_copy(b2c[:], b2p_t[:])
        identc = const.tile([128, 128], f32)
        nc.vector.tensor_copy(identc[:], ident_t[:])
        if M2_MODE == "f32r":
            dwt2r_t = const.tile([128, 128], f32r)
            nc.vector.tensor_copy(dwt2r_t[:], dwt2_t[:])
        else:
            dwt2r_t = dwt2_t

        energy = [epool.tile([128, bpc], f32, tag=f"energy{g}", name=f"energy{g}") for g in range(2)]

        xp = [None] * npairs
        for b in range(bpc):
            pair, b2 = divmod(b, 2)
            if b2 == 0:
                xp[pair] = xpool.tile([128, CW], f32, tag="xpair", name=f"xpair{pair}")
            xt = xp[pair]
            half = slice(b2 * 64, b2 * 64 + 64)

            # ---- load this image: [64 h, (c, w)] into its partition half ----
            # single DMA so downstream matmuls wait on one semaphore only
            nc.sync.dma_start(
                xt[half, :].rearrange("h (c w) -> h c w", w=64),
                xin[b].rearrange("c h w -> h c w"),
            )

            for g in range(2):
                # ---- M1: A^T for 64 channel-pairs of this group ----
                # at_sb free layout: pair p at [p*64, p*64+64), value A^T[w, i]
                # partition layout: member m at [m*64, m*64+64) (m = c % 2)
                at_tiles = []
                at_dt = f32r if M2_MODE == "f32r" else f32
                for htile in range(2):  # 32 pairs per at tile
                    at = atsb.tile([128, 2048], at_dt, tag="at")
                    at_tiles.append(at)
                    for pc in range(4):  # 8 pairs per psum tile
                        aps = at_ps.tile([128, 512], f32, tag="atps")
                        for pp in range(8):
                            p = htile * 32 + pc * 8 + pp
                            c0 = g * 128 + 2 * p
                            nc.tensor.matmul(
                                aps[:, pp * 64 : (pp + 1) * 64],
                                lhsT=xt[half, c0 * 64 : (c0 + 2) * 64],
                                rhs=dht2c[half, :],
                                start=True,
                                stop=True,
                            )
                        nc.vector.tensor_copy(
                            at[:, pc * 512 : (pc + 1) * 512], aps[:]
                        )

                # ---- M2 + |.| eviction + flatten (via DRAM bounce) ----
                # scr layout: [m, j, p, i]; hop1 writes yab stream
                # ((m,j) part, (p,i) free) contiguously; hop2 reads back
                # permuted (p, j, i) into channel-major flat rows.
                fl = flatp.tile([128, 4096], flat_dt, tag="flat")
                scr = scrp.tile([2, 64, 64, 64], flat_dt, tag="scr")
                for htile in range(2):
                    at = at_tiles[htile]
                    yab = ypool.tile([128, 2048], flat_dt, tag="yab")
                    for ch in range(4):
                        yps = y_ps.tile([128, 512], f32, tag="yps")
                        lhs_ap = dwt2r_t[:]
                        rhs_ap = at[:, ch * 512 : (ch + 1) * 512]
                        nc.tensor.matmul(
                            yps[:], lhsT=lhs_ap, rhs=rhs_ap, start=True, stop=True
                        )
                        nc.scalar.activation(
                            yab[:, ch * 512 : (ch + 1) * 512], yps[:], AF.Abs
                        )
                    nc.sync.dma_start(
                        scr[:, :, htile * 32 : (htile + 1) * 32, :], yab[:]
                    )
                for m in range(2):
                    nc.sync.dma_start(
                        fl[m * 64 : (m + 1) * 64, :].rearrange(
                            "p (j i) -> p j i", j=64
                        ),
                        scr[m].rearrange("j p i -> p j i"),
                    )

                # ---- top-k energy ----
                t8 = small.tile([128, 8], flat_dt, tag="top8")
                nc.vector.max(out=t8[:], in_=fl[:, 1:4096])
                ecol = energy[g][:, b : b + 1]
                nc.vector.reduce_sum(
                    out=ecol, in_=t8[:, 0:4], axis=mybir.AxisListType.X
                )
                dc32 = small.tile([128, 1], f32, tag="dc32")
                nc.vector.tensor_copy(dc32[:], fl[:, 0:1])
                nc.vector.tensor_add(ecol, ecol, dc32[:])

            # ---- MLP (per image) ----
            hps = s_ps.tile([CH, 1], f32, tag="sps")
            nc.tensor.matmul(
                hps[:], lhsT=w1c[:, 0:CH], rhs=energy[0][:, b : b + 1],
                start=True, stop=False,
            )
            nc.tensor.matmul(
                hps[:], lhsT=w1c[:, CH : 2 * CH], rhs=energy[1][:, b : b + 1],
                start=False, stop=False,
            )
            nc.tensor.matmul(
                hps[:], lhsT=b1c[:], rhs=ones_t[:, 0:1], start=False, stop=True
            )
            hid = small.tile([CH, 1], f32, tag="hid")
            nc.scalar.activation(hid[:], hps[:], AF.Relu)

            arow_ps = s_ps.tile([1, 256], f32, tag="sps")
            for g in range(2):
                aps2 = s_ps.tile([128, 1], f32, tag="sps")
                nc.tensor.matmul(
                    aps2[:], lhsT=w2c[:, g * 128 : (g + 1) * 128], rhs=hid[:],
                    start=True, stop=False,
                )
                nc.tensor.matmul(
                    aps2[:], lhsT=b2c[:, g * 128 : (g + 1) * 128],
                    rhs=ones_t[:, 0:1], start=False, stop=True,
                )
                att = small.tile([128, 1], f32, tag="att")
                nc.scalar.activation(att[:], aps2[:], AF.Sigmoid)
                nc.tensor.transpose(
                    arow_ps[0:1, g * 128 : (g + 1) * 128], att[:], identc[:]
                )
            arow = small.tile([1, 256], f32, tag="arow")
            nc.scalar.copy(arow[:], arow_ps[:])

            # ---- broadcast attn to [64 h, 256 c] in true-channel order ----
            bc_ps = s_ps.tile([128, 256], f32, tag="sps")
            rhs_perm = arow[:].rearrange("a (g m p) -> a g p m", g=2, m=2, p=64)
            nc.tensor.matmul(
                bc_ps[half, :], lhsT=ones_t[:, 0:64], rhs=rhs_perm,
                start=True, stop=True,
            )
            attnb = small.tile([128, 256], f32, tag="attnb")
            nc.scalar.copy(attnb[half, :], bc_ps[half, :])

            # ---- multiply + store ----
            for cq in range(4):
                seg = slice(cq * 4096, (cq + 1) * 4096)
                x3 = xt[half, seg].rearrange("h (c w) -> h c w", w=64)
                a3 = attnb[half, cq * 64 : (cq + 1) * 64].unsqueeze(2).to_broadcast(
                    [64, 64, 64]
                )
                nc.vector.tensor_mul(x3, x3, a3)
                nc.sync.dma_start(
                    xout[b, cq * 64 : (cq + 1) * 64].rearrange("c h w -> h c w"),
                    xt[half, seg].rearrange("h (c w) -> h c w", w=64),
                )

    nc.compile()
    return nc


def make_host_inputs():
    """Constant tensors shared by all cores."""
    DH = _dct_matrix(H)
    DW = _dct_matrix(W)
    dht2 = np.zeros((128, 64), np.float32)
    dht2[0:64, :] = DH.T
    dht2[64:128, :] = DH.T
    m2np = np.float32 if M2_MODE != "bf16" else None
    dwt2 = np.zeros((128, 128), np.float32)
    dwt2[0:64, 0:64] = DW.T
    dwt2[64:128, 64:128] = DW.T
    ident = np.eye(128, dtype=np.float32)
    return dht2, dwt2, ident


def make_weight_inputs(w1, b1, w2, b2):
    w1ps = np.zeros((128, 128), np.float32)
    w2p = np.zeros((CH, 256), np.float32)
    b2p = np.zeros((1, 256), np.float32)
    for g in range(2):
        cs = np.array([_perm_true_channel(g, q) for q in range(128)])
        w1ps[:, g * CH : (g + 1) * CH] = w1[cs, :]
        w2p[:, g * 128 : (g + 1) * 128] = w2[:, cs]
        b2p[0, g * 128 : (g + 1) * 128] = b2[cs]
    b1v = b1.reshape(1, CH).astype(np.float32)
    return w1ps, b1v, w2p, b2p


_CACHE = {}


def kernel(x, w1, b1, w2, b2):
    from concourse.bass_utils import run_bass_kernel_spmd

    x = np.asarray(x, dtype=np.float32)
    dht2, dwt2, ident = make_host_inputs()
    if M2_MODE == "bf16":
        import ml_dtypes

        dwt2 = dwt2.astype(ml_dtypes.bfloat16)
    w1ps, b1v, w2p, b2p = make_weight_inputs(
        np.asarray(w1, np.float32),
        np.asarray(b1, np.float32),
        np.asarray(w2, np.float32),
        np.asarray(b2, np.float32),
    )

    if "nc" not in _CACHE:
        _CACHE["nc"] = build_nc(BPC)
    nc = _CACHE["nc"]

    in_maps = []
    for i in range(NCORES):
        in_maps.append(
            {
                "xin": np.ascontiguousarray(x[i * BPC : (i + 1) * BPC]),
                "dht2": dht2,
                "dwt2": dwt2,
                "w1ps": w1ps,
                "b1v": b1v,
                "w2p": w2p,
                "b2p": b2p,
                "ident": ident,
            }
        )

    res = run_bass_kernel_spmd(nc, in_maps, list(range(NCORES)))
    outs = [np.asarray(res.results[i]["xout"]) for i in range(NCORES)]
    return np.concatenate(outs, axis=0).astype(np.float32)


# revision 15
# speedup vs baseline: 10.2907x; 10.2907x over previous
"""DCT channel attention kernel for Trainium2 (8 NeuronCores, data-parallel over batch).

Math (per image b, channel c):
  Y = DH @ X @ DW^T              (2D orthonormal DCT of the 64x64 spatial map)
  energy = |Y[0,0]| + sum(top4(|Y| excluding DC))
  attn = sigmoid(relu(energy @ w1 + b1) @ w2 + b2)
  out = x * attn[:, :, None, None]

On-chip layout strategy per core (4 images = 2 partition-pairs):
  x loaded as [128 = (b2, h), C*W] tiles (free = (c, w), channel-major runs of 256B)
  M1: per channel-pair matmul, lhsT = X-slice [64 h, 128 (2ch, w)], rhs = DH^T
      -> A^T in PSUM [128 = (member, w), 64 i]      (fp32, exact)
  M2: lhsT = block-diag(DW^T, DW^T) [128, 128] stationary, rhs = A^T chunks
      -> Y^T [128 = (member, j), (pair, i)] in PSUM (float32r, 1 cyc/row)
  evict |Y| as bf16 (ScalarE Abs), DMA-flatten to [128 = channel, 4096 = (j, i)],
  DVE max (top-8 per partition) -> energy -> tiny MLP on PE -> attn broadcast via
  ones-outer-product matmul -> in-place DVE multiply -> store.

Channel permutation: flat row q within group g of 128 channels maps to true
channel c = g*128 + 2*(q % 64) + (q // 64). MLP weights are permuted host-side;
the broadcast matmul un-permutes via a strided AP.
"""

import numpy as np

B, C, H, W = 32, 256, 64, 64
NCORES = 8
BPC = B // NCORES  # images per core
CW = C * W
RED = 4
CH = C // RED  # 64 hidden units

# dtype for the second DCT matmul: "f32r" (fast, slightly reduced precision),
# "f32" (exact, 4 cyc/row), or "bf16"
M2_MODE = "f32r"
FLAT_BF16 = True


def _dct_matrix(N):
    n = np.arange(N, dtype=np.float64)
    k = np.arange(N, dtype=np.float64)[:, None]
    d = np.cos(np.pi * (2.0 * n + 1.0) * k / (2.0 * N))
    s = np.where(k == 0, np.sqrt(1.0 / N), np.sqrt(2.0 / N))
    return (d * s).astype(np.float32)  # [N, N], D[k, n]


def _perm_true_channel(g, q):
    # flat row q in group g -> true channel index
    return g * 128 + 2 * (q % 64) + (q // 64)


def build_nc(bpc=BPC):
    import concourse.bass as bass
    import concourse.tile as tile
    from concourse import bacc, mybir
    from contextlib import ExitStack

    f32 = mybir.dt.float32
    f32r = mybir.dt.float32r
    bf16 = mybir.dt.bfloat16
    flat_dt = bf16 if FLAT_BF16 else f32

    nc = bacc.Bacc("TRN2", target_bir_lowering=False, debug=False)

    xin = nc.dram_tensor("xin", [bpc, C, H, W], f32, kind="ExternalInput").ap()
    dht2_d = nc.dram_tensor("dht2", [128, 64], f32, kind="ExternalInput").ap()
    m2dt = {"f32r": f32, "f32": f32, "bf16": bf16}[M2_MODE]
    dwt2_d = nc.dram_tensor("dwt2", [128, 128], m2dt, kind="ExternalInput").ap()
    w1ps_d = nc.dram_tensor("w1ps", [128, 128], f32, kind="ExternalInput").ap()
    b1_d = nc.dram_tensor("b1v", [1, CH], f32, kind="ExternalInput").ap()
    w2p_d = nc.dram_tensor("w2p", [CH, 256], f32, kind="ExternalInput").ap()
    b2p_d = nc.dram_tensor("b2p", [1, 256], f32, kind="ExternalInput").ap()
    ident_d = nc.dram_tensor("ident", [128, 128], f32, kind="ExternalInput").ap()
    xout = nc.dram_tensor("xout", [bpc, C, H, W], f32, kind="ExternalOutput").ap()

    AF = mybir.ActivationFunctionType
    npairs = 2 if bpc > 2 else 1

    with tile.TileContext(nc) as tc, ExitStack() as ctx:
        const = ctx.enter_context(tc.tile_pool(name="const", bufs=1))
        xpool = ctx.enter_context(tc.tile_pool(name="xp", bufs=npairs))
        atsb = ctx.enter_context(tc.tile_pool(name="atsb", bufs=3))
        ypool = ctx.enter_context(tc.tile_pool(name="yab", bufs=2))
        flatp = ctx.enter_context(tc.tile_pool(name="flat", bufs=2))
        small = ctx.enter_context(tc.tile_pool(name="small", bufs=2))
        epool = ctx.enter_context(tc.tile_pool(name="energy", bufs=1))
        at_ps = ctx.enter_context(tc.tile_pool(name="atps", bufs=3, space="PSUM"))
        y_ps = ctx.enter_context(tc.tile_pool(name="yps", bufs=2, space="PSUM"))
        s_ps = ctx.enter_context(tc.tile_pool(name="sps", bufs=2, space="PSUM"))
        scrp = ctx.enter_context(tc.tile_pool(name="scr", bufs=2, space="DRAM"))

        # ---- constants ----
        dht2_t = const.tile([128, 64], f32)
        nc.sync.dma_start(dht2_t[:], dht2_d[:])
        dwt2_t = const.tile([128, 128], m2dt)
        nc.sync.dma_start(dwt2_t[:], dwt2_d[:])
        w1ps_t = const.tile([128, 128], f32)
        nc.sync.dma_start(w1ps_t[:], w1ps_d[:])
        b1_t = const.tile([1, CH], f32)
        nc.sync.dma_start(b1_t[:], b1_d[:])
        w2p_t = const.tile([CH, 256], f32)
        nc.sync.dma_start(w2p_t[:], w2p_d[:])
        b2p_t = const.tile([1, 256], f32)
        nc.sync.dma_start(b2p_t[:], b2p_d[:])
        ident_t = const.tile([128, 128], f32)
        nc.sync.dma_start(ident_t[:], ident_d[:])
        ones_t = const.tile([1, CH], f32)
        nc.vector.memset(ones_t[:], 1.0)
        # DVE-owned copies of every constant the PE reads: funnels all
        # matmul input deps onto the single DVE semaphore (the fp32
        # matmul LDW micro-op only supports one sync wait command).
        dht2c = const.tile([128, 64], f32)
        nc.vector.tensor_copy(dht2c[:], dht2_t[:])
        w1c = const.tile([128, 128], f32)
        nc.vector.tensor_copy(w1c[:], w1ps_t[:])
        b1c = const.tile([1, CH], f32)
        nc.vector.tensor_copy(b1c[:], b1_t[:])
        w2c = const.tile([CH, 256], f32)
        nc.vector.tensor_copy(w2c[:], w2p_t[:])
        b2c = const.tile([1, 256], f32)
        nc.vector.tensor_copy(b2c[:], b2p_t[:])
        identc = const.tile([128, 128], f32)
        nc.vector.tensor_copy(identc[:], ident_t[:])
        if M2_MODE == "f32r":
            dwt2r_t = const.tile([128, 128], f32r)
            nc.vector.tensor_copy(dwt2r_t[:], dwt2_t[:])
        else:
            dwt2r_t = dwt2_t

        energy = [epool.tile([128, bpc], f32, tag=f"energy{g}", name=f"energy{g}") for g in range(2)]

        xp = [None] * npairs
        for b in range(bpc):
            pair, b2 = divmod(b, 2)
            if b2 == 0:
                xp[pair] = xpool.tile([128, CW], f32, tag="xpair", name=f"xpair{pair}")
            xt = xp[pair]
            half = slice(b2 * 64, b2 * 64 + 64)

            # ---- load this image: [64 h, (c, w)] into its partition half ----
            # single DMA so downstream matmuls wait on one semaphore only
            nc.sync.dma_start(
                xt[half, :].rearrange("h (c w) -> h c w", w=64),
                xin[b].rearrange("c h w -> h c w"),
            )

            for g in range(2):
                # ---- M1: A^T for 64 channel-pairs of this group ----
                # at_sb free layout: pair p at [p*64, p*64+64), value A^T[w, i]
                # partition layout: member m at [m*64, m*64+64) (m = c % 2)
                at_tiles = []
                at_dt = f32r if M2_MODE == "f32r" else f32
                for htile in range(2):  # 32 pairs per at tile
                    at = atsb.tile([128, 2048], at_dt, tag="at")
                    at_tiles.append(at)
                    for pc in range(4):  # 8 pairs per psum tile
                        aps = at_ps.tile([128, 512], f32, tag="atps")
                        for pp in range(8):
                            p = htile * 32 + pc * 8 + pp
                            c0 = g * 128 + 2 * p
                            nc.tensor.matmul(
                                aps[:, pp * 64 : (pp + 1) * 64],
                                lhsT=xt[half, c0 * 64 : (c0 + 2) * 64],
                                rhs=dht2c[half, :],
                                start=True,
                                stop=True,
                            )
                        nc.vector.tensor_copy(
                            at[:, pc * 512 : (pc + 1) * 512], aps[:]
                        )

                # ---- M2 + |.| eviction + flatten (via DRAM bounce) ----
                # scr layout: [m, j, p, i]; hop1 writes yab stream
                # ((m,j) part, (p,i) free) contiguously; hop2 reads back
                # permuted (p, j, i) into channel-major flat rows.
                fl = flatp.tile([128, 4096], flat_dt, tag="flat")
                scr = scrp.tile([2, 64, 64, 64], flat_dt, tag="scr")
                for htile in range(2):
                    at = at_tiles[htile]
                    yab = ypool.tile([128, 2048], flat_dt, tag="yab")
                    for ch in range(4):
                        yps = y_ps.tile([128, 512], f32, tag="yps")
                        lhs_ap = dwt2r_t[:]
                        rhs_ap = at[:, ch * 512 : (ch + 1) * 512]
                        nc.tensor.matmul(
                            yps[:], lhsT=lhs_ap, rhs=rhs_ap, start=True, stop=True
                        )
                        nc.scalar.activation(
                            yab[:, ch * 512 : (ch + 1) * 512], yps[:], AF.Abs
                        )
                    nc.sync.dma_start(
                        scr[:, :, htile * 32 : (htile + 1) * 32, :], yab[:]
                    )
                for m in range(2):
                    nc.sync.dma_start(
                        fl[m * 64 : (m + 1) * 64, :].rearrange(
                            "p (j i) -> p j i", j=64
                        ),
                        scr[m].rearrange("j p i -> p j i"),
                    )

                # ---- top-k energy ----
                t8 = small.tile([128, 8], flat_dt, tag="top8")
                nc.vector.max(out=t8[:], in_=fl[:, 1:4096])
                ecol = energy[g][:, b : b + 1]
                nc.vector.reduce_sum(
                    out=ecol, in_=t8[:, 0:4], axis=mybir.AxisListType.X
                )
                dc32 = small.tile([128, 1], f32, tag="dc32")
                nc.vector.tensor_copy(dc32[:], fl[:, 0:1])
                nc.vector.tensor_add(ecol, ecol, dc32[:])

            # ---- MLP (per image) ----
            hps = s_ps.tile([CH, 1], f32, tag="sps")
            nc.tensor.matmul(
                hps[:], lhsT=w1c[:, 0:CH], rhs=energy[0][:, b : b + 1],
                start=True, stop=False,
            )
            nc.tensor.matmul(
                hps[:], lhsT=w1c[:, CH : 2 * CH], rhs=energy[1][:, b : b + 1],
                start=False, stop=False,
            )
            nc.tensor.matmul(
                hps[:], lhsT=b1c[:], rhs=ones_t[:, 0:1], start=False, stop=True
            )
            hid = small.tile([CH, 1], f32, tag="hid")
            nc.scalar.activation(hid[:], hps[:], AF.Relu)

            arow_ps = s_ps.tile([1, 256], f32, tag="sps")
            for g in range(2):
                aps2 = s_ps.tile([128, 1], f32, tag="sps")
                nc.tensor.matmul(
                    aps2[:], lhsT=w2c[:, g * 128 : (g + 1) * 128], rhs=hid[:],
                    start=True, stop=False,
                )
                nc.tensor.matmul(
                    aps2[:], lhsT=b2c[:, g * 128 : (g + 1) * 128],
                    rhs=ones_t[:, 0:1], start=False, stop=True,
                )
                att = small.tile([128, 1], f32, tag="att")
                nc.scalar.activation(att[:], aps2[:], AF.Sigmoid)
                nc.tensor.transpose(
                    arow_ps[0:1, g * 128 : (g + 1) * 128], att[:], identc[:]
                )
            arow = small.tile([1, 256], f32, tag="arow")
            nc.scalar.copy(arow[:], arow_ps[:])

            # ---- broadcast attn to [64 h, 256 c] in true-channel order ----
            bc_ps = s_ps.tile([128, 256], f32, tag="sps")
            rhs_perm = arow[:].rearrange("a (g m p) -> a g p m", g=2, m=2, p=64)
            nc.tensor.matmul(
                bc_ps[half, :], lhsT=ones_t[:, 0:64], rhs=rhs_perm,
                start=True, stop=True,
            )
            attnb = small.tile([128, 256], f32, tag="attnb")
            nc.scalar.copy(attnb[half, :], bc_ps[half, :])

            # ---- multiply + store ----
            for cq in range(4):
                seg = slice(cq * 4096, (cq + 1) * 4096)
                x3 = xt[half, seg].rearrange("h (c w) -> h c w", w=64)
                a3 = attnb[half, cq * 64 : (cq + 1) * 64].unsqueeze(2).to_broadcast(
                    [64, 64, 64]
                )
                nc.vector.tensor_mul(x3, x3, a3)
                nc.sync.dma_start(
                    xout[b, cq * 64 : (cq + 1) * 64].rearrange("c h w -> h c w"),
                    xt[half, seg].rearrange("h (c w) -> h c w", w=64),
                )

    nc.compile()
    return nc


def make_host_inputs():
    """Constant tensors shared by all cores."""
    DH = _dct_matrix(H)
    DW = _dct_matrix(W)
    dht2 = np.zeros((128, 64), np.float32)
    dht2[0:64, :] = DH.T
    dht2[64:128, :] = DH.T
    m2np = np.float32 if M2_MODE != "bf16" else None
    dwt2 = np.zeros((128, 128), np.float32)
    dwt2[0:64, 0:64] = DW.T
    dwt2[64:128, 64:128] = DW.T
    ident = np.eye(128, dtype=np.float32)
    return dht2, dwt2, ident


def make_weight_inputs(w1, b1, w2, b2):
    w1ps = np.zeros((128, 128), np.float32)
    w2p = np.zeros((CH, 256), np.float32)
    b2p = np.zeros((1, 256), np.float32)
    for g in range(2):
        cs = np.array([_perm_true_channel(g, q) for q in range(128)])
        w1ps[:, g * CH : (g + 1) * CH] = w1[cs, :]
        w2p[:, g * 128 : (g + 1) * 128] = w2[:, cs]
        b2p[0, g * 128 : (g + 1) * 128] = b2[cs]
    b1v = b1.reshape(1, CH).astype(np.float32)
    return w1ps, b1v, w2p, b2p


_CACHE = {}


def _get_runner():
    """Build (once) a cached jitted SPMD executable over 8 cores.

    Mirrors concourse.bass2jax.run_bass_via_pjrt's multi-core path but keeps
    the jitted function alive so repeat invocations skip re-tracing.
    """
    if "runner" in _CACHE:
        return _CACHE["runner"]
    import jax
    from jax.experimental.shard_map import shard_map
    from jax.sharding import Mesh, PartitionSpec
    from concourse import bass2jax, mybir
    from concourse.bass2jax import _bass_exec_p, install_neuronx_cc_hook

    install_neuronx_cc_hook()
    nc = build_nc(BPC)

    partition_name = (
        nc.partition_id_tensor.name if nc.partition_id_tensor else None
    )
    in_names, out_names, out_avals = [], [], []
    for alloc in nc.m.functions[0].allocations:
        if not isinstance(alloc, mybir.MemoryLocationSet):
            continue
        name = alloc.memorylocations[0].name
        if alloc.kind == "ExternalInput":
            if name != partition_name:
                in_names.append(name)
        elif alloc.kind == "ExternalOutput":
            out_names.append(name)
            out_avals.append(
                jax.core.ShapedArray(
                    tuple(alloc.tensor_shape), mybir.dt.np(alloc.dtype)
                )
            )
    n_params = len(in_names)
    all_in_names = in_names + out_names
    if partition_name is not None:
        all_in_names = all_in_names + [partition_name]

    def _body(*args):
        operands = list(args)
        if partition_name is not None:
            operands.append(bass2jax.partition_id_tensor())
        outs = _bass_exec_p.bind(
            *operands,
            out_avals=tuple(out_avals),
            in_names=tuple(all_in_names),
            out_names=tuple(out_names),
            lowering_input_output_aliases=(),
            sim_require_finite=True,
            sim_require_nnan=True,
            nc=nc,
        )
        return tuple(outs)

    devices = jax.devices()[:NCORES]
    mesh = Mesh(np.asarray(devices), ("core",))
    nin = n_params + len(out_names)
    sharded = jax.jit(
        shard_map(
            _body,
            mesh=mesh,
            in_specs=(PartitionSpec("core"),) * nin,
            out_specs=(PartitionSpec("core"),) * len(out_names),
            check_rep=False,
        ),
        donate_argnums=tuple(range(n_params, nin)),
        keep_unused=True,
    )
    runner = (sharded, in_names, out_names, out_avals)
    _CACHE["runner"] = runner
    return runner


def make_concat_inputs(x, w1, b1, w2, b2):
    """Per-core inputs concatenated along axis 0 (shard_map layout)."""
    x = np.asarray(x, dtype=np.float32)
    dht2, dwt2, ident = make_host_inputs()
    w1ps, b1v, w2p, b2p = make_weight_inputs(
        np.asarray(w1, np.float32),
        np.asarray(b1, np.float32),
        np.asarray(w2, np.float32),
        np.asarray(b2, np.float32),
    )
    per_core = {
        "dht2": dht2, "dwt2": dwt2, "w1ps": w1ps, "b1v": b1v,
        "w2p": w2p, "b2p": b2p, "ident": ident,
    }
    vals = {"xin": np.ascontiguousarray(x)}
    for k, v in per_core.items():
        vals[k] = np.concatenate([v] * NCORES, axis=0)
    return vals


def kernel(x, w1, b1, w2, b2):
    sharded, in_names, out_names, out_avals = _get_runner()
    vals = make_concat_inputs(x, w1, b1, w2, b2)
    concat_in = [vals[n] for n in in_names]
    concat_zeros = [
        np.zeros((NCORES * a.shape[0], *a.shape[1:]), a.dtype) for a in out_avals
    ]
    out_arrs = sharded(*concat_in, *concat_zeros)
    return np.asarray(out_arrs[out_names.index("xout")]).astype(np.float32)
